# revision 1
# baseline (speedup 1.0000x reference)
"""Trainium2 Bass kernel for CustomTriangleMultiplicationOutgoing.

Reference computation (B=1, N=384, D=C=128):
    z_norm = LN(z) * g + b                        # over D
    left   = (z_norm@Wa + ba) * sigmoid(z_norm@Wga + bga) * mask
    right  = (z_norm@Wb + bb) * sigmoid(z_norm@Wgb + bgb) * mask
    z_out[i,j,c] = sum_k left[i,k,c] * right[j,k,c]
    z_out  = LN(z_out) * g_out + b_out            # over C
    out    = (z_out@Wo + bo) * sigmoid(z_norm@Wgo + bgo)

Sharding: 1D over the first N (i) axis, 48 rows per core.  Each core
computes its row-shard's projections locally (left + out-gate stay in
SBUF in bf16), the gated right projection is AllGathered in bf16 across
the 8 cores in 4 c-range chunks (pipelined against the einsum), the
einsum runs on the tensor engine with k on partitions, and the final
LN + Wo + gate is fused on the way out.

Engine discipline: ACT runs a single activation function per phase
(batched Sqrt for phase-1 LN, Sigmoid for gates, Sqrt again in phase 3)
to avoid 1.3us activation-table reloads; matmul operands are bf16 so
FWL fast weight loads engage.
"""

import numpy as np
import ml_dtypes

import concourse.bass as bass
import concourse.mybir as mybir
import concourse.tile as tile
from concourse import bacc
from concourse.masks import make_identity
from concourse.bass_utils import run_bass_kernel_spmd

F32 = mybir.dt.float32
BF16 = mybir.dt.bfloat16
EPS = 1e-5

B = 1
N_FULL = 384
D = 128
C = 128
W = 8  # cores
P = 128


def bcast_part(ap, parts):
    """Broadcast a [1, ...] AP across `parts` partitions (partition step 0)."""
    return bass.AP(tensor=ap.tensor, offset=ap.offset, ap=[[0, parts]] + ap.ap[1:])


def build_nc(n=N_FULL, with_bias=False, with_mask=False, nq=4, cb=8):
    """Build the SPMD Bass program (same program on all 8 cores)."""
    assert n % P == 0 and n % W == 0
    SH = n // W          # rows of i per core
    KC = n // P          # 128-wide chunks of k
    NT = SH * n // P     # phase-1 tiles per core (= SH*KC)
    CQ = C // nq         # c per AllGather chunk
    assert CQ % cb == 0

    nc = bacc.Bacc(None, num_devices=W)

    z_sh = nc.declare_dram_parameter("z_sh", [SH * n, D], BF16, isOutput=False)
    wcat = nc.declare_dram_parameter("wcat", [D, 4 * C], BF16, isOutput=False)
    wgo = nc.declare_dram_parameter("wgo", [D, D], BF16, isOutput=False)
    wo = nc.declare_dram_parameter("wo", [C, D + 1], BF16, isOutput=False)
    wosum = nc.declare_dram_parameter("wosum", [1, D], F32, isOutput=False)
    if with_bias:
        bcat = nc.declare_dram_parameter("bcat", [1, 4 * C], F32, isOutput=False)
        bgo_p = nc.declare_dram_parameter("bgo", [1, D], F32, isOutput=False)
        bo_p = nc.declare_dram_parameter("bo", [1, D], F32, isOutput=False)
    if with_mask:
        mask_sh = nc.declare_dram_parameter("mask_sh", [SH * n], F32, isOutput=False)
    out_sh = nc.declare_dram_parameter("out_sh", [SH * n, D], F32, isOutput=True)

    # internal DRAM
    right_q = [nc.dram_tensor(f"right_{q}", [KC, P, CQ, SH], BF16) for q in range(nq)]
    gath_q = [
        nc.dram_tensor(f"gath_{q}", [W, KC, P, CQ, SH], BF16, addr_space="Shared")
        for q in range(nq)
    ]
    zout = nc.dram_tensor("zout", [C, SH, n], BF16)  # c-major einsum result

    with tile.TileContext(nc) as tc:
        with tc.tile_pool(name="singles", bufs=1) as singles:
            ident = singles.tile([P, P], BF16)
            make_identity(nc, ident)
            wcat_sb = singles.tile([D, 4 * C], BF16)
            nc.sync.dma_start(wcat_sb, wcat[:])
            wgo_sb = singles.tile([D, D], BF16)
            nc.sync.dma_start(wgo_sb, wgo[:])
            wo_sb = singles.tile([C, D + 1], BF16)
            nc.sync.dma_start(wo_sb, wo[:])
            wosum_b = singles.tile([P, D], F32)
            nc.sync.dma_start(wosum_b, bcast_part(wosum[:], P))
            ones_bf = singles.tile([C, 1], BF16)
            nc.vector.memset(ones_bf, 1.0)
            eps_sb = singles.tile([P, 1], F32)
            nc.vector.memset(eps_sb, EPS)
            if with_bias:
                bcat_sb = singles.tile([P, 4 * C], F32)
                nc.sync.dma_start(bcat_sb, bcast_part(bcat[:], P))
                bgo_sb = singles.tile([P, D], F32)
                nc.sync.dma_start(bgo_sb, bcast_part(bgo_p[:], P))
                bo_sb = singles.tile([P, D], F32)
                nc.sync.dma_start(bo_sb, bcast_part(bo_p[:], P))
            if with_mask:
                mask_sb = singles.tile([P, NT], F32)
                nc.sync.dma_start(
                    mask_sb, mask_sh[:].rearrange("(t p) -> p t", p=P)
                )

            # persistent bf16 stores
            L_sb = singles.tile([P, KC, SH, C], BF16)   # left:  [k, kc, i_loc, c]
            gg_sb = singles.tile([P, NT, D], BF16)      # out-gate per row tile

            # ---------------- phase 1: LN + projections ----------------
            with (
                tc.tile_pool(name="p1_z", bufs=1) as zpool,
                tc.tile_pool(name="p1_temps", bufs=4) as temps,
                tc.tile_pool(name="p1_stats", bufs=1) as spool,
                tc.tile_pool(name="p1_psum", bufs=2, space="PSUM") as psum1,
                tc.tile_pool(name="p1_rstage", bufs=1) as rstage,
            ):
                zbuf = zpool.tile([P, NT, D], BF16)
                nc.sync.dma_start(zbuf, z_sh[:].rearrange("(t p) d -> p t d", p=P))
                # batched LN stats: one Sqrt for all tiles
                mv_all = spool.tile([P, NT, 2], F32)
                for t in range(NT):
                    stats = temps.tile([P, 6], F32, tag="stats")
                    nc.vector.bn_stats(stats, zbuf[:, t, :])
                    nc.vector.bn_aggr(mv_all[:, t, :], stats)
                rstd_all = spool.tile([P, NT], F32)
                nc.scalar.activation(
                    rstd_all, mv_all[:, :, 1],
                    mybir.ActivationFunctionType.Sqrt, bias=eps_sb,
                )
                nc.vector.reciprocal(rstd_all, rstd_all)

                R_stage = rstage.tile([P, KC, C, SH], BF16)  # right: [k, kc, c, j_loc]
                for t in range(NT):
                    i_loc = t // KC
                    kc = t % KC
                    xn = temps.tile([P, D], BF16, tag="xn")
                    nc.vector.tensor_scalar(
                        xn,
                        zbuf[:, t, :],
                        scalar1=mv_all[:, t, 0:1],
                        scalar2=rstd_all[:, t : t + 1],
                        op0=mybir.AluOpType.subtract,
                        op1=mybir.AluOpType.mult,
                    )
                    # transpose z_norm tile -> T [D, rows]
                    pt = psum1.tile([P, P], BF16, tag="pt")
                    nc.tensor.transpose(pt, xn, ident)
                    T = temps.tile([P, P], BF16, tag="T")
                    nc.vector.tensor_copy(T, pt)
                    # projections: [rows, a|ga|b|gb] and [rows, go]
                    p1 = psum1.tile([P, 4 * C], F32, tag="p1")
                    nc.tensor.matmul(p1, lhsT=T, rhs=wcat_sb, start=True, stop=True)
                    p2 = psum1.tile([P, D], F32, tag="p2")
                    nc.tensor.matmul(p2, lhsT=T, rhs=wgo_sb, start=True, stop=True)
                    if with_bias:
                        nc.vector.tensor_tensor(p1, p1, bcat_sb, mybir.AluOpType.add)
                        nc.vector.tensor_tensor(p2, p2, bgo_sb, mybir.AluOpType.add)
                    # gates (ACT: Sigmoid only)
                    sga = temps.tile([P, C], F32, tag="sga")
                    nc.scalar.activation(
                        sga, p1[:, C : 2 * C], mybir.ActivationFunctionType.Sigmoid
                    )
                    sgb = temps.tile([P, C], F32, tag="sgb")
                    nc.scalar.activation(
                        sgb, p1[:, 3 * C : 4 * C], mybir.ActivationFunctionType.Sigmoid
                    )
                    if with_mask:
                        nc.gpsimd.tensor_scalar_mul(sga, sga, mask_sb[:, t : t + 1])
                        nc.gpsimd.tensor_scalar_mul(sgb, sgb, mask_sb[:, t : t + 1])
                    # out-gate (sigmoid straight to bf16 store)
                    nc.scalar.activation(
                        gg_sb[:, t, :], p2, mybir.ActivationFunctionType.Sigmoid
                    )
                    # gated products into einsum-layout stores
                    nc.vector.tensor_tensor(
                        L_sb[:, kc, i_loc, :], p1[:, 0:C], sga, mybir.AluOpType.mult
                    )
                    nc.vector.tensor_tensor(
                        R_stage[:, kc, :, i_loc],
                        p1[:, 2 * C : 3 * C],
                        sgb,
                        mybir.AluOpType.mult,
                    )
                # write right shard to DRAM AllGather inputs
                for q in range(nq):
                    for kc in range(KC):
                        nc.sync.dma_start(
                            right_q[q][kc],
                            R_stage[:, kc, q * CQ : (q + 1) * CQ, :],
                        )

            # ---------------- phase 1.5: AllGather right ----------------
            for q in range(nq):
                nc.gpsimd.collective_compute(
                    "AllGather",
                    mybir.AluOpType.bypass,
                    replica_groups=[list(range(W))],
                    ins=[right_q[q][:]],
                    outs=[gath_q[q][:]],
                )

            # ---------------- phase 2: einsum ----------------
            with (
                tc.tile_pool(name="p2_r", bufs=2) as rpool,
                tc.tile_pool(name="p2_st", bufs=4) as stpool,
                tc.tile_pool(name="p2_psum", bufs=8, space="PSUM") as psum2,
            ):
                for q in range(nq):
                    for cbi in range(CQ // cb):
                        R_blk = rpool.tile([P, KC, W, cb, SH], BF16, tag="rblk")
                        for kc in range(KC):
                            for m in range(W):
                                nc.sync.dma_start(
                                    R_blk[:, kc, m],
                                    gath_q[q][m, kc, :, cbi * cb : (cbi + 1) * cb, :],
                                )
                        for c_ in range(cb):
                            c_glob = q * CQ + cbi * cb + c_
                            ps = psum2.tile([SH, n], F32, tag="ps")
                            for kc in range(KC):
                                nc.tensor.matmul(
                                    ps,
                                    lhsT=L_sb[:, kc, :, c_glob],
                                    rhs=R_blk[:, kc, :, c_, :],
                                    start=(kc == 0),
                                    stop=(kc == KC - 1),
                                )
                            st = stpool.tile([SH, n], BF16, tag="st")
                            if c_ % 2 == 0:
                                nc.vector.tensor_copy(st, ps)
                            else:
                                nc.scalar.copy(st, ps)
                            nc.sync.dma_start(zout[c_glob], st)

            # ---------------- phase 3: LN(z_out) @ Wo * gate ----------------
            zout_flat = zout[:].rearrange("c i j -> c (i j)")
            with (
                tc.tile_pool(name="p3_temps", bufs=4) as t3,
                tc.tile_pool(name="p3_psum", bufs=2, space="PSUM") as psum3,
            ):
                for rt in range(NT):
                    zt = t3.tile([C, P], BF16, tag="zt")
                    nc.sync.dma_start(zt, zout_flat[:, P * rt : P * (rt + 1)])
                    sq = t3.tile([C, P], BF16, tag="sq")
                    nc.vector.tensor_tensor(sq, zt, zt, mybir.AluOpType.mult)
                    # pr[:, 0:D] = Z.T @ Wo' ; pr[:, D] = per-row sum of Z (ones col)
                    pr = psum3.tile([P, D + 1], F32, tag="pr")
                    nc.tensor.matmul(pr, lhsT=zt, rhs=wo_sb, start=True, stop=True)
                    ss = psum3.tile([P, 1], F32, tag="ss")
                    nc.tensor.matmul(ss, lhsT=sq, rhs=ones_bf, start=True, stop=True)
                    mean = t3.tile([P, 1], F32, tag="mean")
                    nc.vector.tensor_scalar_mul(mean, pr[:, D : D + 1], 1.0 / C)
                    msq = t3.tile([P, 1], F32, tag="msq")
                    nc.vector.tensor_scalar_mul(msq, ss, 1.0 / C)
                    var = t3.tile([P, 1], F32, tag="var")
                    nc.vector.tensor_tensor(var, mean, mean, mybir.AluOpType.mult)
                    nc.vector.tensor_tensor(var, msq, var, mybir.AluOpType.subtract)
                    rstd3 = t3.tile([P, 1], F32, tag="rstd3")
                    nc.scalar.activation(
                        rstd3, var, mybir.ActivationFunctionType.Sqrt, bias=eps_sb
                    )
                    nc.vector.reciprocal(rstd3, rstd3)
                    sc = t3.tile([P, 1], F32, tag="sc")
                    nc.vector.tensor_tensor(sc, mean, rstd3, mybir.AluOpType.mult)
                    # out = rstd*(Z.T@Wo) - (rstd*mean)*colsum(Wo)  [+ bo]
                    corr = t3.tile([P, D], F32, tag="corr")
                    nc.vector.tensor_scalar_mul(corr, wosum_b, sc)
                    po = t3.tile([P, D], F32, tag="po")
                    nc.vector.tensor_scalar_mul(po, pr[:, 0:D], rstd3)
                    nc.vector.tensor_tensor(po, po, corr, mybir.AluOpType.subtract)
                    if with_bias:
                        nc.vector.tensor_tensor(po, po, bo_sb, mybir.AluOpType.add)
                    ot = t3.tile([P, D], F32, tag="ot")
                    nc.vector.tensor_tensor(
                        ot, po, gg_sb[:, rt, :], mybir.AluOpType.mult
                    )
                    nc.sync.dma_start(out_sh[P * rt : P * (rt + 1), :], ot)

    nc.compile()
    return nc


_CACHE = {}


def _get_nc(n, with_bias, with_mask):
    key = (n, with_bias, with_mask)
    if key not in _CACHE:
        _CACHE[key] = build_nc(n=n, with_bias=with_bias, with_mask=with_mask)
    return _CACHE[key]


def prepare_host(z, mask, norm_g, norm_b, norm_out_g, norm_out_b,
                 Wa, ba, Wb, bb, Wga, bga, Wgb, bgb, Wo, bo, Wgo, bgo, n=N_FULL):
    """Fold norm affines into weights; build per-core input maps."""
    f = np.asarray
    z = f(z, dtype=np.float32)
    mask = f(mask, dtype=np.float32)
    g = f(norm_g, np.float32)
    b = f(norm_b, np.float32)
    go = f(norm_out_g, np.float32)
    bo_n = f(norm_out_b, np.float32)

    def fold(Wm, bias):
        Wm = f(Wm, np.float32)
        bias = f(bias, np.float32)
        return g[:, None] * Wm, bias + b @ Wm

    Wa_, ba_ = fold(Wa, ba)
    Wga_, bga_ = fold(Wga, bga)
    Wb_, bb_ = fold(Wb, bb)
    Wgb_, bgb_ = fold(Wgb, bgb)
    Wgo_, bgo_ = fold(Wgo, bgo)
    Wo_ = go[:, None] * f(Wo, np.float32)
    bo_ = f(bo, np.float32) + bo_n @ f(Wo, np.float32)

    bf = ml_dtypes.bfloat16
    wcat = np.concatenate([Wa_, Wga_, Wb_, Wgb_], axis=1).astype(bf)
    woa = np.concatenate([Wo_, np.ones((C, 1), np.float32)], axis=1).astype(bf)
    wosum_h = Wo_.sum(axis=0)[None, :].astype(np.float32)
    bcat = np.concatenate([ba_, bga_, bb_, bgb_])[None, :].astype(np.float32)

    with_bias = bool(np.any(bcat) or np.any(bgo_) or np.any(bo_))
    with_mask = not bool(np.all(mask == 1.0))

    SH = n // W
    in_maps = []
    for m in range(W):
        im = {
            "z_sh": np.ascontiguousarray(
                z[0, SH * m : SH * (m + 1)].reshape(SH * n, D)
            ).astype(bf),
            "wcat": wcat,
            "wgo": np.ascontiguousarray(Wgo_).astype(bf),
            "wo": woa,
            "wosum": wosum_h,
        }
        if with_bias:
            im["bcat"] = bcat
            im["bgo"] = bgo_[None, :].astype(np.float32)
            im["bo"] = bo_[None, :].astype(np.float32)
        if with_mask:
            im["mask_sh"] = np.ascontiguousarray(
                mask[0, SH * m : SH * (m + 1)].reshape(SH * n)
            )
        in_maps.append(im)
    return in_maps, with_bias, with_mask


def kernel(**inputs):
    n = inputs["z"].shape[1]
    in_maps, with_bias, with_mask = prepare_host(**inputs, n=n)
    nc = _get_nc(n, with_bias, with_mask)
    res = run_bass_kernel_spmd(nc, in_maps, list(range(W)))
    SH = n // W
    parts = [res.results[m]["out_sh"].reshape(SH, n, D) for m in range(W)]
    out = np.concatenate(parts, axis=0)[None]  # [1, n, n, D]
    return out.astype(np.float32)



# revision 3
# speedup vs baseline: 1.0482x; 1.0482x over previous
"""Trainium2 Bass kernel for CustomTriangleMultiplicationOutgoing.

Reference computation (B=1, N=384, D=C=128):
    z_norm = LN(z) * g + b                        # over D
    left   = (z_norm@Wa + ba) * sigmoid(z_norm@Wga + bga) * mask
    right  = (z_norm@Wb + bb) * sigmoid(z_norm@Wgb + bgb) * mask
    z_out[i,j,c] = sum_k left[i,k,c] * right[j,k,c]
    z_out  = LN(z_out) * g_out + b_out            # over C
    out    = (z_out@Wo + bo) * sigmoid(z_norm@Wgo + bgo)

Sharding: 1D over the first N (i) axis, 48 rows per core.  Each core
computes its row-shard's projections locally (left + out-gate stay in
SBUF in bf16), the gated right projection is AllGathered in bf16 across
the 8 cores in 4 c-range chunks (pipelined against the einsum), the
einsum runs on the tensor engine with k on partitions, and the final
LN + Wo + gate is fused on the way out.

Engine discipline: ACT runs a single activation function per phase
(batched Sqrt for phase-1 LN, Sigmoid for gates, Sqrt again in phase 3)
to avoid 1.3us activation-table reloads; matmul operands are bf16 so
FWL fast weight loads engage.
"""

import numpy as np
import ml_dtypes

import concourse.bass as bass
import concourse.mybir as mybir
import concourse.tile as tile
from concourse import bacc
from concourse.masks import make_identity
from concourse.bass_utils import run_bass_kernel_spmd

F32 = mybir.dt.float32
BF16 = mybir.dt.bfloat16
EPS = 1e-5

B = 1
N_FULL = 384
D = 128
C = 128
W = 8  # cores
P = 128


def bcast_part(ap, parts):
    """Broadcast a [1, ...] AP across `parts` partitions (partition step 0)."""
    return bass.AP(tensor=ap.tensor, offset=ap.offset, ap=[[0, parts]] + ap.ap[1:])


def build_nc(n=N_FULL, with_bias=False, with_mask=False, nq=4, cb=16):
    """Build the SPMD Bass program (same program on all 8 cores)."""
    assert n % P == 0 and n % W == 0
    SH = n // W          # rows of i per core
    KC = n // P          # 128-wide chunks of k
    NT = SH * n // P     # phase-1 tiles per core (= SH*KC)
    CQ = C // nq         # c per AllGather chunk
    assert CQ % cb == 0

    nc = bacc.Bacc(None, num_devices=W)

    z_sh = nc.declare_dram_parameter("z_sh", [SH * n, D], BF16, isOutput=False)
    wcat = nc.declare_dram_parameter("wcat", [D, 4 * C], BF16, isOutput=False)
    wgo = nc.declare_dram_parameter("wgo", [D, D], BF16, isOutput=False)
    wo = nc.declare_dram_parameter("wo", [C, D + 1], BF16, isOutput=False)
    wosum = nc.declare_dram_parameter("wosum", [1, D], F32, isOutput=False)
    if with_bias:
        bcat = nc.declare_dram_parameter("bcat", [1, 4 * C], F32, isOutput=False)
        bgo_p = nc.declare_dram_parameter("bgo", [1, D], F32, isOutput=False)
        bo_p = nc.declare_dram_parameter("bo", [1, D], F32, isOutput=False)
    if with_mask:
        mask_sh = nc.declare_dram_parameter("mask_sh", [SH * n], F32, isOutput=False)
    out_sh = nc.declare_dram_parameter("out_sh", [SH * n, D], F32, isOutput=True)

    # internal DRAM
    right_q = [nc.dram_tensor(f"right_{q}", [KC, P, CQ, SH], BF16) for q in range(nq)]
    gath_q = [
        nc.dram_tensor(f"gath_{q}", [W, KC, P, CQ, SH], BF16, addr_space="Shared")
        for q in range(nq)
    ]
    zout = nc.dram_tensor("zout", [C, SH, n], BF16)  # c-major einsum result

    with tile.TileContext(nc) as tc:
        with tc.tile_pool(name="singles", bufs=1) as singles:
            ident = singles.tile([P, P], BF16)
            make_identity(nc, ident)
            wcat_sb = singles.tile([D, 4 * C], BF16)
            nc.sync.dma_start(wcat_sb, wcat[:])
            wgo_sb = singles.tile([D, D], BF16)
            nc.sync.dma_start(wgo_sb, wgo[:])
            wo_sb = singles.tile([C, D + 1], BF16)
            nc.sync.dma_start(wo_sb, wo[:])
            wosum_b = singles.tile([P, D], F32)
            nc.sync.dma_start(wosum_b, bcast_part(wosum[:], P))
            ones_bf = singles.tile([C, 1], BF16)
            nc.vector.memset(ones_bf, 1.0)
            eps_sb = singles.tile([P, 1], F32)
            nc.vector.memset(eps_sb, EPS)
            if with_bias:
                bcat_sb = singles.tile([P, 4 * C], F32)
                nc.sync.dma_start(bcat_sb, bcast_part(bcat[:], P))
                bgo_sb = singles.tile([P, D], F32)
                nc.sync.dma_start(bgo_sb, bcast_part(bgo_p[:], P))
                bo_sb = singles.tile([P, D], F32)
                nc.sync.dma_start(bo_sb, bcast_part(bo_p[:], P))
            if with_mask:
                mask_sb = singles.tile([P, NT], F32)
                nc.sync.dma_start(
                    mask_sb, mask_sh[:].rearrange("(t p) -> p t", p=P)
                )

            # persistent bf16 stores
            L_sb = singles.tile([P, KC, SH, C], BF16)   # left:  [k, kc, i_loc, c]
            gg_sb = singles.tile([P, NT, D], BF16)      # out-gate per row tile

            # ---------------- phase 1: LN + projections ----------------
            with (
                tc.tile_pool(name="p1_z", bufs=1) as zpool,
                tc.tile_pool(name="p1_temps", bufs=4) as temps,
                tc.tile_pool(name="p1_stats", bufs=1) as spool,
                tc.tile_pool(name="p1_psum", bufs=2, space="PSUM") as psum1,
                tc.tile_pool(name="p1_rstage", bufs=1) as rstage,
            ):
                zbuf = zpool.tile([P, NT, D], BF16)
                nc.sync.dma_start(zbuf, z_sh[:].rearrange("(t p) d -> p t d", p=P))
                # batched LN stats: one Sqrt for all tiles
                mv_all = spool.tile([P, NT, 2], F32)
                for t in range(NT):
                    stats = temps.tile([P, 6], F32, tag="stats")
                    nc.vector.bn_stats(stats, zbuf[:, t, :])
                    nc.vector.bn_aggr(mv_all[:, t, :], stats)
                rstd_all = spool.tile([P, NT], F32)
                nc.scalar.activation(
                    rstd_all, mv_all[:, :, 1],
                    mybir.ActivationFunctionType.Sqrt, bias=eps_sb,
                )
                nc.vector.reciprocal(rstd_all, rstd_all)

                R_stage = rstage.tile([P, KC, C, SH], BF16)  # right: [k, kc, c, j_loc]
                for t in range(NT):
                    i_loc = t // KC
                    kc = t % KC
                    xn = temps.tile([P, D], BF16, tag="xn")
                    nc.vector.tensor_scalar(
                        xn,
                        zbuf[:, t, :],
                        scalar1=mv_all[:, t, 0:1],
                        scalar2=rstd_all[:, t : t + 1],
                        op0=mybir.AluOpType.subtract,
                        op1=mybir.AluOpType.mult,
                    )
                    # transpose z_norm tile -> T [D, rows]
                    pt = psum1.tile([P, P], BF16, tag="pt")
                    nc.tensor.transpose(pt, xn, ident)
                    T = temps.tile([P, P], BF16, tag="T")
                    nc.vector.tensor_copy(T, pt)
                    # projections: [rows, a|ga|b|gb] and [rows, go]
                    p1 = psum1.tile([P, 4 * C], F32, tag="p1")
                    nc.tensor.matmul(p1, lhsT=T, rhs=wcat_sb, start=True, stop=True)
                    p2 = psum1.tile([P, D], F32, tag="p2")
                    nc.tensor.matmul(p2, lhsT=T, rhs=wgo_sb, start=True, stop=True)
                    if with_bias:
                        nc.vector.tensor_tensor(p1, p1, bcat_sb, mybir.AluOpType.add)
                        nc.vector.tensor_tensor(p2, p2, bgo_sb, mybir.AluOpType.add)
                    # gates (ACT: Sigmoid only)
                    sga = temps.tile([P, C], F32, tag="sga")
                    nc.scalar.activation(
                        sga, p1[:, C : 2 * C], mybir.ActivationFunctionType.Sigmoid
                    )
                    sgb = temps.tile([P, C], F32, tag="sgb")
                    nc.scalar.activation(
                        sgb, p1[:, 3 * C : 4 * C], mybir.ActivationFunctionType.Sigmoid
                    )
                    if with_mask:
                        nc.gpsimd.tensor_scalar_mul(sga, sga, mask_sb[:, t : t + 1])
                        nc.gpsimd.tensor_scalar_mul(sgb, sgb, mask_sb[:, t : t + 1])
                    # out-gate (sigmoid straight to bf16 store)
                    nc.scalar.activation(
                        gg_sb[:, t, :], p2, mybir.ActivationFunctionType.Sigmoid
                    )
                    # gated products into einsum-layout stores
                    nc.vector.tensor_tensor(
                        L_sb[:, kc, i_loc, :], p1[:, 0:C], sga, mybir.AluOpType.mult
                    )
                    nc.vector.tensor_tensor(
                        R_stage[:, kc, :, i_loc],
                        p1[:, 2 * C : 3 * C],
                        sgb,
                        mybir.AluOpType.mult,
                    )
                # write right shard to DRAM AllGather inputs
                for q in range(nq):
                    for kc in range(KC):
                        nc.sync.dma_start(
                            right_q[q][kc],
                            R_stage[:, kc, q * CQ : (q + 1) * CQ, :],
                        )

            # ---------------- phase 1.5: AllGather right ----------------
            for q in range(nq):
                nc.gpsimd.collective_compute(
                    "AllGather",
                    mybir.AluOpType.bypass,
                    replica_groups=[list(range(W))],
                    ins=[right_q[q][:]],
                    outs=[gath_q[q][:]],
                )

            # ---------------- phase 2: einsum ----------------
            with (
                tc.tile_pool(name="p2_r", bufs=2) as rpool,
                tc.tile_pool(name="p2_st", bufs=4) as stpool,
                tc.tile_pool(name="p2_psum", bufs=8, space="PSUM") as psum2,
            ):
                for q in range(nq):
                    for cbi in range(CQ // cb):
                        R_blk = rpool.tile([P, KC, W, cb, SH], BF16, tag="rblk")
                        for kc in range(KC):
                            for m in range(W):
                                nc.sync.dma_start(
                                    R_blk[:, kc, m],
                                    gath_q[q][m, kc, :, cbi * cb : (cbi + 1) * cb, :],
                                )
                        for c_ in range(cb):
                            c_glob = q * CQ + cbi * cb + c_
                            ps = psum2.tile([SH, n], F32, tag="ps")
                            for kc in range(KC):
                                nc.tensor.matmul(
                                    ps,
                                    lhsT=L_sb[:, kc, :, c_glob],
                                    rhs=R_blk[:, kc, :, c_, :],
                                    start=(kc == 0),
                                    stop=(kc == KC - 1),
                                )
                            st = stpool.tile([SH, n], BF16, tag="st")
                            if c_ % 2 == 0:
                                nc.vector.tensor_copy(st, ps)
                            else:
                                nc.scalar.copy(st, ps)
                            nc.sync.dma_start(zout[c_glob], st)

            # ---------------- phase 3: LN(z_out) @ Wo * gate ----------------
            zout_flat = zout[:].rearrange("c i j -> c (i j)")
            with (
                tc.tile_pool(name="p3_temps", bufs=4) as t3,
                tc.tile_pool(name="p3_psum", bufs=2, space="PSUM") as psum3,
            ):
                for rt4 in range(NT // 4):
                    zt4 = t3.tile([C, 4 * P], BF16, tag="zt4")
                    nc.sync.dma_start(
                        zt4, zout_flat[:, 4 * P * rt4 : 4 * P * (rt4 + 1)]
                    )
                    ot4 = t3.tile([P, 4, D], F32, tag="ot4")
                    for u in range(4):
                        rt = rt4 * 4 + u
                        zt = zt4[:, u * P : (u + 1) * P]
                        sq = t3.tile([C, P], BF16, tag="sq")
                        nc.vector.tensor_tensor(sq, zt, zt, mybir.AluOpType.mult)
                        # pr[:, 0:D] = Z.T @ Wo' ; pr[:, D] = row sum of Z
                        pr = psum3.tile([P, D + 1], F32, tag="pr")
                        nc.tensor.matmul(
                            pr, lhsT=zt, rhs=wo_sb, start=True, stop=True
                        )
                        ss = psum3.tile([P, 1], F32, tag="ss")
                        nc.tensor.matmul(
                            ss, lhsT=sq, rhs=ones_bf, start=True, stop=True
                        )
                        mean = t3.tile([P, 1], F32, tag="mean")
                        nc.vector.tensor_scalar_mul(mean, pr[:, D : D + 1], 1.0 / C)
                        msq = t3.tile([P, 1], F32, tag="msq")
                        nc.vector.tensor_scalar_mul(msq, ss, 1.0 / C)
                        var = t3.tile([P, 1], F32, tag="var")
                        nc.vector.tensor_tensor(var, mean, mean, mybir.AluOpType.mult)
                        nc.vector.tensor_tensor(
                            var, msq, var, mybir.AluOpType.subtract
                        )
                        rstd3 = t3.tile([P, 1], F32, tag="rstd3")
                        nc.scalar.activation(
                            rstd3, var, mybir.ActivationFunctionType.Sqrt,
                            bias=eps_sb,
                        )
                        nc.vector.reciprocal(rstd3, rstd3)
                        sc = t3.tile([P, 1], F32, tag="sc")
                        nc.vector.tensor_tensor(sc, mean, rstd3, mybir.AluOpType.mult)
                        # out = rstd*(Z.T@Wo) - (rstd*mean)*colsum(Wo)  [+ bo]
                        corr = t3.tile([P, D], F32, tag="corr")
                        nc.vector.tensor_scalar_mul(corr, wosum_b, sc)
                        po = t3.tile([P, D], F32, tag="po")
                        nc.vector.tensor_scalar_mul(po, pr[:, 0:D], rstd3)
                        nc.vector.tensor_tensor(po, po, corr, mybir.AluOpType.subtract)
                        if with_bias:
                            nc.vector.tensor_tensor(
                                po, po, bo_sb, mybir.AluOpType.add
                            )
                        nc.vector.tensor_tensor(
                            ot4[:, u, :], po, gg_sb[:, rt, :], mybir.AluOpType.mult
                        )
                    nc.sync.dma_start(
                        out_sh[4 * P * rt4 : 4 * P * (rt4 + 1), :].rearrange(
                            "(t p) d -> p t d", p=P
                        ),
                        ot4,
                    )

    nc.compile()
    return nc


_CACHE = {}


def _get_nc(n, with_bias, with_mask):
    key = (n, with_bias, with_mask)
    if key not in _CACHE:
        _CACHE[key] = build_nc(n=n, with_bias=with_bias, with_mask=with_mask)
    return _CACHE[key]


def prepare_host(z, mask, norm_g, norm_b, norm_out_g, norm_out_b,
                 Wa, ba, Wb, bb, Wga, bga, Wgb, bgb, Wo, bo, Wgo, bgo, n=N_FULL):
    """Fold norm affines into weights; build per-core input maps."""
    f = np.asarray
    z = f(z, dtype=np.float32)
    mask = f(mask, dtype=np.float32)
    g = f(norm_g, np.float32)
    b = f(norm_b, np.float32)
    go = f(norm_out_g, np.float32)
    bo_n = f(norm_out_b, np.float32)

    def fold(Wm, bias):
        Wm = f(Wm, np.float32)
        bias = f(bias, np.float32)
        return g[:, None] * Wm, bias + b @ Wm

    Wa_, ba_ = fold(Wa, ba)
    Wga_, bga_ = fold(Wga, bga)
    Wb_, bb_ = fold(Wb, bb)
    Wgb_, bgb_ = fold(Wgb, bgb)
    Wgo_, bgo_ = fold(Wgo, bgo)
    Wo_ = go[:, None] * f(Wo, np.float32)
    bo_ = f(bo, np.float32) + bo_n @ f(Wo, np.float32)

    bf = ml_dtypes.bfloat16
    wcat = np.concatenate([Wa_, Wga_, Wb_, Wgb_], axis=1).astype(bf)
    woa = np.concatenate([Wo_, np.ones((C, 1), np.float32)], axis=1).astype(bf)
    wosum_h = Wo_.sum(axis=0)[None, :].astype(np.float32)
    bcat = np.concatenate([ba_, bga_, bb_, bgb_])[None, :].astype(np.float32)

    with_bias = bool(np.any(bcat) or np.any(bgo_) or np.any(bo_))
    with_mask = not bool(np.all(mask == 1.0))

    SH = n // W
    in_maps = []
    for m in range(W):
        im = {
            "z_sh": np.ascontiguousarray(
                z[0, SH * m : SH * (m + 1)].reshape(SH * n, D)
            ).astype(bf),
            "wcat": wcat,
            "wgo": np.ascontiguousarray(Wgo_).astype(bf),
            "wo": woa,
            "wosum": wosum_h,
        }
        if with_bias:
            im["bcat"] = bcat
            im["bgo"] = bgo_[None, :].astype(np.float32)
            im["bo"] = bo_[None, :].astype(np.float32)
        if with_mask:
            im["mask_sh"] = np.ascontiguousarray(
                mask[0, SH * m : SH * (m + 1)].reshape(SH * n)
            )
        in_maps.append(im)
    return in_maps, with_bias, with_mask


def kernel(**inputs):
    n = inputs["z"].shape[1]
    in_maps, with_bias, with_mask = prepare_host(**inputs, n=n)
    nc = _get_nc(n, with_bias, with_mask)
    res = run_bass_kernel_spmd(nc, in_maps, list(range(W)))
    SH = n // W
    parts = [res.results[m]["out_sh"].reshape(SH, n, D) for m in range(W)]
    out = np.concatenate(parts, axis=0)[None]  # [1, n, n, D]
    return out.astype(np.float32)

